# revision 21
# baseline (speedup 1.0000x reference)
"""MoE-routed DeepQNetwork kernel for 8x Trainium2 NeuronCores.

Problem: B=65536 rows, each routed to one of E=8 expert MLPs
(256 -> 64 -> 64 -> 64 -> 64 -> 64 -> 18, ReLU between layers).

Strategy (expert == core):
  E equals NCORES, so pad every expert's row group to the same block count
  nb = ceil(max_count/512) and give core c exactly expert c's rows. Every
  core then runs one expert's weights for its whole batch slice:

  - weights per core shrink to a single static [128, 676] fp16 tile
    (W1 as two 128-row chunks, W2..W5 as 128x128 block-diagonal with the
    SAME expert in both halves, W6 block-diag at 36 columns), loaded once;
  - 512-row blocks run in pairs stacked on the 128 partitions; pairs are
    fused 2-at-a-time into [128, 1024] PSUM tiles so one Vector/Scalar
    activation op covers two pairs (PSUM-access overhead amortized);
  - layer-6 output lands in a [36, 512] PSUM tile (rows 0:18 block a,
    18:36 block b) and is DMAed straight to DRAM with an fp32->fp16 cast
    on the GpSimd SWDGE queue - no activation op, no SBUF staging
    (b6 is added on the host in the rare case it is nonzero);
  - x arrives as [128, 2C] fp16 (two 128-dim chunks per pair side by
    side), split into per-pair DMAs issued in the order the wavefront
    consumes them; compute is emitted in a diagonal (unit, layer)
    wavefront so the in-order Tensor stream never camps on a late DMA.

  Host: stable-sort rows by expert, pad, transpose into the device layout;
  unsort the [36, cols] outputs back to the original row order.
"""

import math
import os

import numpy as np

E = 8
D = 256
H = 64
A = 18
NCORES = 8
BLK = 512

# wt column layout (fp16, [128, 676]):
#   [0:64)    W1 chunk0 (x dims 0:128)
#   [64:128)  W1 chunk1 (x dims 128:256)
#   [128+128*li : 256+128*li) for li in 0..3: W_{li+2} block-diag
#             ([0:64, 0:64] = W, [64:128, 64:128] = W)
#   [640:676) W6 block-diag ([0:64, 0:18] = W6, [64:128, 18:36] = W6)
WT_COLS = 676
W6C = 640

_PROGRAM_CACHE: dict = {}
LAST_RESULTS = None  # test harness can read timing/profile info from here


def _make_units(npair: int, nlone: int):
    """Unit schedule: lone block first (its x chunk is smallest and issued
    first), then one unit per pair. Small units maximize the number of
    independent (unit, layer) chains in flight so the in-order Tensor
    stream never waits long on an activation and the PE p-state ramps."""
    units = []
    if nlone:
        units.append(("lone", None))
    for p in range(npair):
        units.append(("pairs", [p]))
    return units


def _build_program(nb: int):
    import concourse.mybir as mybir
    import concourse.tile as tile
    from concourse import bacc

    f32 = mybir.dt.float32
    f16 = mybir.dt.float16
    Relu = mybir.ActivationFunctionType.Relu
    add = mybir.AluOpType.add
    amax = mybir.AluOpType.max

    npair = nb // 2
    nlone = nb % 2
    NU = npair + nlone
    C = nb * BLK

    nc = bacc.Bacc("TRN2")
    xt = nc.declare_dram_parameter("xt", [128, 2 * C], f16, isOutput=False)
    wt = nc.declare_dram_parameter("wt", [128, WT_COLS], f16, isOutput=False)
    bias = nc.declare_dram_parameter("bias", [128, 6], f32, isOutput=False)
    yt = nc.declare_dram_parameter("yt", [36, NU * BLK], f16, isOutput=True)

    units = _make_units(npair, nlone)
    NUNITS = len(units)
    # output store groups: contiguous runs of units sharing one SBUF o tile
    # and a single SP-ring store each (unit u owns yt cols [512u, 512u+512));
    # the final group is a single unit so the last store drains fast
    if NUNITS >= 5:
        gsplit = [0, (NUNITS - 1) // 3, 2 * (NUNITS - 1) // 3, NUNITS - 1, NUNITS]
    else:
        gsplit = list(range(NUNITS + 1))
    ngrp = len(gsplit) - 1
    grp_of = {}
    for g in range(ngrp):
        for u in range(gsplit[g], gsplit[g + 1]):
            grp_of[u] = g

    with tile.TileContext(nc) as tc:
        with (
            tc.tile_pool(name="wpool", bufs=1) as wpool,
            tc.tile_pool(name="xpool", bufs=1) as xpool,
            tc.tile_pool(name="hpool", bufs=1) as hpool,
            tc.tile_pool(name="opool", bufs=1) as opool,
            tc.tile_pool(name="ppool", bufs=1, space="PSUM") as ppool,
        ):
            wt_sb = wpool.tile([128, WT_COLS], f16, name="wt_sb", tag="wt")
            bias_sb = wpool.tile([128, 6], f32, name="bias_sb", tag="bias")
            nc.scalar.dma_start(out=wt_sb[:, :], in_=wt[:, :])
            nc.scalar.dma_start(out=bias_sb[:, :], in_=bias[:, :])

            # PE p-state warm-up: the PE clock ramps 0.65 -> 1.2 -> 2.4 GHz
            # only under ~3us of continuous execution. Run dummy matmuls on a
            # zeroed tile while the first x chunk is still in flight so real
            # matmuls start at full clock.
            wu = wpool.tile([128, 512], f16, name="wu", tag="wu")
            nc.gpsimd.memset(wu[:, :], 0.0)
            pwu = ppool.tile([64, 512], f32, tag="pwu", name="pwu", bufs=1)
            for i in range(11):
                nc.tensor.matmul(
                    out=pwu[:, :],
                    lhsT=wu[:, 0:64],
                    rhs=wu[:, :],
                    start=True,
                    stop=True,
                )

            # x chunks all ride the SP HWDGE ring in consumption order: the
            # ring delivers completions in issue order with low latency,
            # while chunks scattered to the Act/SWDGE rings were measured to
            # complete ~9us later than same-position SP-ring chunks
            xl = None
            if nlone:
                xl = xpool.tile([128, 1024], f16, name="x_lone", tag="xl")
                nc.sync.dma_start(
                    out=xl[:, :], in_=xt[:, 2048 * npair : 2048 * npair + 1024]
                )
            xcs = []
            for p in range(npair):
                xc = xpool.tile([128, 2048], f16, name=f"x_{p}", tag="xc", bufs=npair)
                nc.sync.dma_start(out=xc[:, :], in_=xt[:, 2048 * p : 2048 * p + 2048])
                xcs.append(xc)

            otiles = [
                opool.tile(
                    [36, 512 * (gsplit[g + 1] - gsplit[g])],
                    f16,
                    name=f"o_{g}",
                    tag=f"o{g}",
                )
                for g in range(ngrp)
            ]

            # Arrival-aware wavefront over (unit, stage). The Tensor stream
            # executes in emission order, so a stage emitted before its input
            # exists blocks everything behind it (head-of-line). Greedily
            # emit: L1 of unit u only once the emitted PE work covers its x
            # chunk's estimated DMA arrival; between L1s emit later stages
            # whose activation (est. ~1us after the producing matmul) is done.
            MM = 0.27  # est. PE us per 512-col matmul (p-state blend)
            ACT = 1.1  # est. matmul-end -> h-ready latency (act + 2 sems)
            arr = [0.0] + [0.9 + 1.31 * (p + 1) for p in range(npair)]
            if not nlone:
                arr = arr[1:]
            stage_mm = lambda un, kk: (
                (2 if units[un][0] == "lone" else 4) if kk == 0 else 1
            )
            nxt = [0] * NUNITS  # next stage to emit per unit
            hready = [0.0] * NUNITS  # est. time unit's last act completes
            pe_t = 0.0
            moves = []
            while any(nk < 6 for nk in nxt):
                best_key, best = None, None
                for un in range(NUNITS):
                    kk = nxt[un]
                    if kk >= 6:
                        continue
                    need = arr[un] if kk == 0 else hready[un]
                    start = max(pe_t, need)
                    # prefer the move that can start earliest; tie-break on
                    # lower stage (breadth-first keeps more chains alive)
                    key = (start, kk, un)
                    if best_key is None or key < best_key:
                        best_key, best = key, (un, kk)
                un, kk = best
                dur = stage_mm(un, kk) * MM
                hready[un] = best_key[0] + dur + ACT
                pe_t = best_key[0] + dur
                nxt[un] += 1
                moves.append((0, kk, un))
            hcur: dict = {}
            for _key, k, u in moves:
                kind, pairs = units[u]
                lone = kind == "lone"
                pr_all = slice(0, 64) if lone else slice(0, 128)
                ps = ppool.tile([128, 512], f32, tag="ps", name=f"ps_{u}_{k}", bufs=7)
                if k == 0:
                    if lone:
                        for c in (0, 1):
                            nc.tensor.matmul(
                                out=ps[0:64, :],
                                lhsT=wt_sb[:, 64 * c : 64 * c + 64],
                                rhs=xl[:, 512 * c : 512 * c + 512],
                                start=(c == 0),
                                stop=(c == 1),
                            )
                    else:
                        xc = xcs[pairs[0]]
                        for blk in (0, 1):
                            pr = slice(64 * blk, 64 * blk + 64)
                            for c in (0, 1):
                                nc.tensor.matmul(
                                    out=ps[pr, :],
                                    lhsT=wt_sb[:, 64 * c : 64 * c + 64],
                                    rhs=xc[
                                        :,
                                        1024 * c + 512 * blk : 1024 * c
                                        + 512 * blk
                                        + 512,
                                    ],
                                    start=(c == 0),
                                    stop=(c == 1),
                                )
                elif k < 5:
                    wc = 128 * k
                    if lone:
                        nc.tensor.matmul(
                            out=ps[0:64, :],
                            lhsT=wt_sb[0:64, wc : wc + 64],
                            rhs=hcur[u][0:64, :],
                            start=True,
                            stop=True,
                        )
                    else:
                        nc.tensor.matmul(
                            out=ps[:, :],
                            lhsT=wt_sb[:, wc : wc + 128],
                            rhs=hcur[u][:, :],
                            start=True,
                            stop=True,
                        )
                else:
                    if lone:
                        nc.tensor.matmul(
                            out=ps[0:18, :],
                            lhsT=wt_sb[0:64, W6C : W6C + 18],
                            rhs=hcur[u][0:64, :],
                            start=True,
                            stop=True,
                        )
                    else:
                        nc.tensor.matmul(
                            out=ps[0:36, :],
                            lhsT=wt_sb[:, W6C : W6C + 36],
                            rhs=hcur[u][:, :],
                            start=True,
                            stop=True,
                        )

                on_scalar = (u + k) % 2 == 0
                if k < 5:
                    h = hpool.tile(
                        [128, 512] if not lone else [64, 512],
                        f16,
                        tag="h" if not lone else "hlone",
                        name=f"h_{u}_{k}",
                        bufs=8 if not lone else 4,
                    )
                    bap = bias_sb[pr_all, k : k + 1]
                    if on_scalar:
                        nc.scalar.activation(h[:, :], ps[pr_all, :], Relu, bias=bap)
                    else:
                        nc.vector.tensor_scalar(
                            h[:, :], ps[pr_all, :], bap, 0.0, op0=add, op1=amax
                        )
                    hcur[u] = h
                else:
                    g = grp_of[u]
                    rows = 18 if lone else 36
                    ocol = slice(512 * (u - gsplit[g]), 512 * (u - gsplit[g]) + 512)
                    ot = otiles[g]
                    b6ap = bias_sb[0:rows, 5:6]
                    if on_scalar:
                        nc.scalar.add(ot[0:rows, ocol], ps[0:rows, :], b6ap)
                    else:
                        nc.vector.tensor_scalar(
                            ot[0:rows, ocol], ps[0:rows, :], b6ap, None, op0=add
                        )
                    if u == gsplit[g + 1] - 1:
                        nc.sync.dma_start(
                            out=yt[:, BLK * gsplit[g] : BLK * gsplit[g + 1]],
                            in_=ot[:, :],
                        )

    nc.compile()
    return nc


def _get_program(nb: int):
    if nb not in _PROGRAM_CACHE:
        _PROGRAM_CACHE[nb] = _build_program(nb)
    return _PROGRAM_CACHE[nb]


def _prepare(state, rm_state, Ws, bs):
    X = np.ascontiguousarray(np.asarray(state, dtype=np.float32)).reshape(-1, D)
    rm = np.asarray(rm_state).reshape(-1).astype(np.int64)
    B = X.shape[0]

    counts = np.bincount(rm, minlength=E)
    nb = max(int(math.ceil(counts.max() / BLK)), 1)
    npair = nb // 2
    nlone = nb % 2
    C = nb * BLK

    order = np.argsort(rm, kind="stable")
    csum = np.zeros(E + 1, dtype=np.int64)
    csum[1:] = np.cumsum(counts)
    Xs = X[order].astype(np.float16)

    W16 = [np.asarray(w, dtype=np.float32).astype(np.float16) for w in Ws]
    bsf = [np.asarray(b, dtype=np.float32) for b in bs]

    in_maps = []
    for e in range(E):
        S = np.zeros((C, D), np.float16)
        S[: counts[e]] = Xs[csum[e] : csum[e + 1]]
        xtc = np.empty((128, 2 * C), np.float16)
        if npair:
            P2 = S[: 1024 * npair].reshape(npair, 2, 512, 2, 128)
            # cols = p*2048 + chunk*1024 + blk*512 + row ; rows = d
            xtc[:, : 2048 * npair] = (
                P2.transpose(4, 0, 3, 1, 2).reshape(128, 2048 * npair)
            )
        if nlone:
            L = S[1024 * npair : 1024 * npair + 512].reshape(512, 2, 128)
            xtc[:, 2048 * npair :] = L.transpose(2, 1, 0).reshape(128, 1024)

        wh = np.zeros((128, WT_COLS), np.float16)
        wh[:, 0:64] = W16[0][e, 0:128, :]
        wh[:, 64:128] = W16[0][e, 128:256, :]
        for li in range(4):
            wc = 128 + 128 * li
            wh[0:64, wc : wc + 64] = W16[li + 1][e]
            wh[64:128, wc + 64 : wc + 128] = W16[li + 1][e]
        wh[0:64, W6C : W6C + A] = W16[5][e]
        wh[64:128, W6C + A : W6C + 2 * A] = W16[5][e]

        bh = np.zeros((128, 6), np.float32)
        for li in range(5):
            bh[0:64, li] = bsf[li][e]
            bh[64:128, li] = bsf[li][e]
        bh[0:A, 5] = bsf[5][e]
        bh[A : 2 * A, 5] = bsf[5][e]

        in_maps.append({"xt": xtc, "wt": wh, "bias": bh})

    meta = dict(
        B=B, nb=nb, npair=npair, nlone=nlone, C=C, counts=counts, csum=csum,
        order=order,
    )
    return in_maps, meta


def _finalize(results, meta):
    B, npair, nlone, C = (meta[k] for k in ("B", "npair", "nlone", "C"))
    counts, csum, order = meta["counts"], meta["csum"], meta["order"]
    y = np.empty((B, A), np.float32)
    for e in range(E):
        ytc = results[e]["yt"].astype(np.float32)  # [36, NU*512]; unit u at 512u
        rows = np.empty((C, A), np.float32)
        if npair:
            yp = ytc[:, 512 * nlone :].reshape(2, A, npair, 512)
            rows[: 1024 * npair] = (
                yp.transpose(2, 0, 3, 1).reshape(1024 * npair, A)
            )
        if nlone:
            rows[1024 * npair : 1024 * npair + 512] = ytc[0:A, 0:512].T
        y[order[csum[e] : csum[e + 1]]] = rows[: counts[e]]
    return y


def kernel(state, rm_state, W1, b1, W2, b2, W3, b3, W4, b4, W5, b5, W6, b6):
    global LAST_RESULTS
    from concourse.bass_utils import run_bass_kernel_spmd

    in_maps, meta = _prepare(
        state, rm_state, (W1, W2, W3, W4, W5, W6), (b1, b2, b3, b4, b5, b6)
    )
    nc = _get_program(meta["nb"])
    trace = bool(os.environ.get("KERNEL_TRACE"))
    res = run_bass_kernel_spmd(nc, in_maps, core_ids=list(range(NCORES)), trace=trace)
    LAST_RESULTS = res
    return _finalize(res.results, meta)


# revision 25
# speedup vs baseline: 1.1477x; 1.1477x over previous
"""MoE-routed DeepQNetwork kernel for 8x Trainium2 NeuronCores.

Problem: B=65536 rows, each routed to one of E=8 expert MLPs
(256 -> 64 -> 64 -> 64 -> 64 -> 64 -> 18, ReLU between layers).

Strategy (expert == core):
  E equals NCORES, so pad every expert's row group to the same block count
  nb = ceil(max_count/512) and give core c exactly expert c's rows. Every
  core then runs one expert's weights for its whole batch slice:

  - weights per core shrink to a single static [128, 676] fp16 tile
    (W1 as two 128-row chunks, W2..W5 as 128x128 block-diagonal with the
    SAME expert in both halves, W6 block-diag at 36 columns), loaded once;
  - 512-row blocks run in pairs stacked on the 128 partitions; pairs are
    fused 2-at-a-time into [128, 1024] PSUM tiles so one Vector/Scalar
    activation op covers two pairs (PSUM-access overhead amortized);
  - layer-6 output lands in a [36, 512] PSUM tile (rows 0:18 block a,
    18:36 block b) and is DMAed straight to DRAM with an fp32->fp16 cast
    on the GpSimd SWDGE queue - no activation op, no SBUF staging
    (b6 is added on the host in the rare case it is nonzero);
  - x arrives as [128, 2C] fp16 (two 128-dim chunks per pair side by
    side), split into per-pair DMAs issued in the order the wavefront
    consumes them; compute is emitted in a diagonal (unit, layer)
    wavefront so the in-order Tensor stream never camps on a late DMA.

  Host: stable-sort rows by expert, pad, transpose into the device layout;
  unsort the [36, cols] outputs back to the original row order.
"""

import math
import os

import numpy as np

E = 8
D = 256
H = 64
A = 18
NCORES = 8
BLK = 512

# wt column layout (fp16, [128, 676]):
#   [0:64)    W1 chunk0 (x dims 0:128)
#   [64:128)  W1 chunk1 (x dims 128:256)
#   [128+128*li : 256+128*li) for li in 0..3: W_{li+2} block-diag
#             ([0:64, 0:64] = W, [64:128, 64:128] = W)
#   [640:676) W6 block-diag ([0:64, 0:18] = W6, [64:128, 18:36] = W6)
WT_COLS = 676
W6C = 640

_PROGRAM_CACHE: dict = {}
LAST_RESULTS = None  # test harness can read timing/profile info from here


def _make_units(npair: int, nlone: int):
    """Unit schedule: lone block first (its x chunk is smallest and issued
    first), then one unit per pair. Small units maximize the number of
    independent (unit, layer) chains in flight so the in-order Tensor
    stream never waits long on an activation and the PE p-state ramps."""
    units = []
    if nlone:
        units.append(("lone", None))
    for p in range(npair):
        units.append(("pairs", [p]))
    return units


def _build_program(nb: int):
    import concourse.mybir as mybir
    import concourse.tile as tile
    from concourse import bacc

    f32 = mybir.dt.float32
    f16 = mybir.dt.float16
    Relu = mybir.ActivationFunctionType.Relu
    add = mybir.AluOpType.add
    amax = mybir.AluOpType.max

    npair = nb // 2
    nlone = nb % 2
    NU = npair + nlone
    C = nb * BLK

    nc = bacc.Bacc("TRN2")
    xt = nc.declare_dram_parameter("xt", [128, 2 * C], f16, isOutput=False)
    wt = nc.declare_dram_parameter("wt", [128, WT_COLS], f16, isOutput=False)
    bias = nc.declare_dram_parameter("bias", [128, 6], f32, isOutput=False)
    yt = nc.declare_dram_parameter("yt", [36, NU * BLK], f16, isOutput=True)

    units = _make_units(npair, nlone)
    NUNITS = len(units)
    # output store groups: contiguous runs of units sharing one SBUF o tile
    # and a single SP-ring store each (unit u owns yt cols [512u, 512u+512));
    # the final group is a single unit so the last store drains fast
    if NUNITS >= 5:
        gsplit = [0, (NUNITS - 1) // 3, 2 * (NUNITS - 1) // 3, NUNITS - 1, NUNITS]
    else:
        gsplit = list(range(NUNITS + 1))
    ngrp = len(gsplit) - 1
    grp_of = {}
    for g in range(ngrp):
        for u in range(gsplit[g], gsplit[g + 1]):
            grp_of[u] = g

    with tile.TileContext(nc) as tc:
        with (
            tc.tile_pool(name="wpool", bufs=1) as wpool,
            tc.tile_pool(name="xpool", bufs=1) as xpool,
            tc.tile_pool(name="hpool", bufs=1) as hpool,
            tc.tile_pool(name="opool", bufs=1) as opool,
            tc.tile_pool(name="ppool", bufs=1, space="PSUM") as ppool,
        ):
            wt_sb = wpool.tile([128, WT_COLS], f16, name="wt_sb", tag="wt")
            bias_sb = wpool.tile([128, 6], f32, name="bias_sb", tag="bias")
            nc.scalar.dma_start(out=wt_sb[:, :], in_=wt[:, :])
            nc.scalar.dma_start(out=bias_sb[:, :], in_=bias[:, :])

            # PE p-state warm-up: the PE clock ramps 0.65 -> 1.2 -> 2.4 GHz
            # only under ~3us of continuous execution. Run dummy matmuls on a
            # zeroed tile while the first x chunk is still in flight so real
            # matmuls start at full clock.
            wu = wpool.tile([128, 512], f16, name="wu", tag="wu")
            nc.gpsimd.memset(wu[:, :], 0.0)
            pwu = ppool.tile([64, 512], f32, tag="pwu", name="pwu", bufs=1)

            def warm_mm():
                nc.tensor.matmul(
                    out=pwu[:, :],
                    lhsT=wu[:, 0:64],
                    rhs=wu[:, :],
                    start=True,
                    stop=True,
                )

            # x chunks all ride the SP HWDGE ring in consumption order: the
            # ring delivers completions in issue order with low latency,
            # while chunks scattered to the Act/SWDGE rings were measured to
            # complete ~9us later than same-position SP-ring chunks
            xl = None
            if nlone:
                xl = xpool.tile([128, 1024], f16, name="x_lone", tag="xl")
                nc.sync.dma_start(
                    out=xl[:, :], in_=xt[:, 2048 * npair : 2048 * npair + 1024]
                )
            xcs = []
            for p in range(npair):
                xc = xpool.tile([128, 2048], f16, name=f"x_{p}", tag="xc", bufs=npair)
                nc.sync.dma_start(out=xc[:, :], in_=xt[:, 2048 * p : 2048 * p + 2048])
                xcs.append(xc)

            otiles = [
                opool.tile(
                    [36, 512 * (gsplit[g + 1] - gsplit[g])],
                    f16,
                    name=f"o_{g}",
                    tag=f"o{g}",
                )
                for g in range(ngrp)
            ]

            # Arrival-aware wavefront over (unit, stage). The Tensor stream
            # executes in emission order, so a stage emitted before its input
            # exists blocks everything behind it (head-of-line). Greedily
            # emit, in estimated absolute time (us from kernel start): L1 of
            # unit u only once its x chunk has landed, later stages once their
            # activation is done, and dependency-free warm-up matmuls into
            # any gap so the PE p-state clock never drops.
            MM = 0.22  # PE us per 512-col matmul at full clock
            SEM = 0.12
            T0 = 7.4  # Tensor stream start (after preamble + wu memset)
            arr = [10.9] + [12.0 + 1.31 * p for p in range(npair)]
            if not nlone:
                arr = arr[1:]
            stage_mm = lambda un, kk: (
                (2 if units[un][0] == "lone" else 4) if kk == 0 else 1
            )
            act_us = lambda un, kk: 0.58 if units[un][0] != "lone" else 0.54
            nxt = [0] * NUNITS
            hready = [0.0] * NUNITS  # est. time unit's h (act output) lands
            eng_free = [0.0, 0.0]  # scalar, vector
            ring = []  # act-done times of live ps ring slots
            pe_t = T0
            warm = 0.0  # time spent warming (clock ramp credit)
            moves = []
            act_eng: dict = {}
            while any(nk < 6 for nk in nxt):
                best_key, best = None, None
                for un in range(NUNITS):
                    kk = nxt[un]
                    if kk >= 6:
                        continue
                    need = arr[un] if kk == 0 else hready[un]
                    start = max(pe_t, need)
                    if len(ring) >= 7:
                        start = max(start, ring[-7])
                    key = (start, kk, un)
                    if best_key is None or key < best_key:
                        best_key, best = key, (un, kk)
                un, kk = best
                start = best_key[0]
                # fill the stall with warm-up matmuls (clock stays ramped)
                while start - pe_t > 0.30:
                    moves.append((None, None))
                    wdur = 0.45 if (pe_t - T0) - warm * 0.0 < 3.6 else 0.23
                    warm += wdur
                    pe_t += wdur
                dur = stage_mm(un, kk) * (0.45 if pe_t - T0 < 3.6 else MM)
                mm_end = max(pe_t, start) + dur
                eng = 0 if eng_free[0] <= eng_free[1] else 1
                a0 = max(mm_end + SEM, eng_free[eng])
                a1 = a0 + act_us(un, kk) * (1.0 if eng == 0 else 1.16)
                eng_free[eng] = a1
                act_eng[(un, kk)] = eng
                hready[un] = a1 + SEM
                ring.append(a1)
                pe_t = mm_end
                nxt[un] += 1
                moves.append((kk, un))
            hcur: dict = {}
            for k, u in moves:
                if k is None:
                    warm_mm()
                    continue
                kind, pairs = units[u]
                lone = kind == "lone"
                pr_all = slice(0, 64) if lone else slice(0, 128)
                ps = ppool.tile([128, 512], f32, tag="ps", name=f"ps_{u}_{k}", bufs=7)
                if k == 0:
                    if lone:
                        for c in (0, 1):
                            nc.tensor.matmul(
                                out=ps[0:64, :],
                                lhsT=wt_sb[:, 64 * c : 64 * c + 64],
                                rhs=xl[:, 512 * c : 512 * c + 512],
                                start=(c == 0),
                                stop=(c == 1),
                            )
                    else:
                        xc = xcs[pairs[0]]
                        for blk in (0, 1):
                            pr = slice(64 * blk, 64 * blk + 64)
                            for c in (0, 1):
                                nc.tensor.matmul(
                                    out=ps[pr, :],
                                    lhsT=wt_sb[:, 64 * c : 64 * c + 64],
                                    rhs=xc[
                                        :,
                                        1024 * c + 512 * blk : 1024 * c
                                        + 512 * blk
                                        + 512,
                                    ],
                                    start=(c == 0),
                                    stop=(c == 1),
                                )
                elif k < 5:
                    wc = 128 * k
                    if lone:
                        nc.tensor.matmul(
                            out=ps[0:64, :],
                            lhsT=wt_sb[0:64, wc : wc + 64],
                            rhs=hcur[u][0:64, :],
                            start=True,
                            stop=True,
                        )
                    else:
                        nc.tensor.matmul(
                            out=ps[:, :],
                            lhsT=wt_sb[:, wc : wc + 128],
                            rhs=hcur[u][:, :],
                            start=True,
                            stop=True,
                        )
                else:
                    if lone:
                        nc.tensor.matmul(
                            out=ps[0:18, :],
                            lhsT=wt_sb[0:64, W6C : W6C + 18],
                            rhs=hcur[u][0:64, :],
                            start=True,
                            stop=True,
                        )
                    else:
                        nc.tensor.matmul(
                            out=ps[0:36, :],
                            lhsT=wt_sb[:, W6C : W6C + 36],
                            rhs=hcur[u][:, :],
                            start=True,
                            stop=True,
                        )

                on_scalar = act_eng[(u, k)] == 0
                if k < 5:
                    h = hpool.tile(
                        [128, 512] if not lone else [64, 512],
                        f16,
                        tag="h" if not lone else "hlone",
                        name=f"h_{u}_{k}",
                        bufs=8 if not lone else 4,
                    )
                    bap = bias_sb[pr_all, k : k + 1]
                    if on_scalar:
                        nc.scalar.activation(h[:, :], ps[pr_all, :], Relu, bias=bap)
                    else:
                        nc.vector.tensor_scalar(
                            h[:, :], ps[pr_all, :], bap, 0.0, op0=add, op1=amax
                        )
                    hcur[u] = h
                else:
                    g = grp_of[u]
                    rows = 18 if lone else 36
                    ocol = slice(512 * (u - gsplit[g]), 512 * (u - gsplit[g]) + 512)
                    ot = otiles[g]
                    b6ap = bias_sb[0:rows, 5:6]
                    if on_scalar:
                        nc.scalar.add(ot[0:rows, ocol], ps[0:rows, :], b6ap)
                    else:
                        nc.vector.tensor_scalar(
                            ot[0:rows, ocol], ps[0:rows, :], b6ap, None, op0=add
                        )
                    if u == gsplit[g + 1] - 1:
                        nc.sync.dma_start(
                            out=yt[:, BLK * gsplit[g] : BLK * gsplit[g + 1]],
                            in_=ot[:, :],
                        )

    nc.compile()
    return nc


def _get_program(nb: int):
    if nb not in _PROGRAM_CACHE:
        _PROGRAM_CACHE[nb] = _build_program(nb)
    return _PROGRAM_CACHE[nb]


def _prepare(state, rm_state, Ws, bs):
    X = np.ascontiguousarray(np.asarray(state, dtype=np.float32)).reshape(-1, D)
    rm = np.asarray(rm_state).reshape(-1).astype(np.int64)
    B = X.shape[0]

    counts = np.bincount(rm, minlength=E)
    nb = max(int(math.ceil(counts.max() / BLK)), 1)
    npair = nb // 2
    nlone = nb % 2
    C = nb * BLK

    order = np.argsort(rm, kind="stable")
    csum = np.zeros(E + 1, dtype=np.int64)
    csum[1:] = np.cumsum(counts)
    Xs = X[order].astype(np.float16)

    W16 = [np.asarray(w, dtype=np.float32).astype(np.float16) for w in Ws]
    bsf = [np.asarray(b, dtype=np.float32) for b in bs]

    in_maps = []
    for e in range(E):
        S = np.zeros((C, D), np.float16)
        S[: counts[e]] = Xs[csum[e] : csum[e + 1]]
        xtc = np.empty((128, 2 * C), np.float16)
        if npair:
            P2 = S[: 1024 * npair].reshape(npair, 2, 512, 2, 128)
            # cols = p*2048 + chunk*1024 + blk*512 + row ; rows = d
            xtc[:, : 2048 * npair] = (
                P2.transpose(4, 0, 3, 1, 2).reshape(128, 2048 * npair)
            )
        if nlone:
            L = S[1024 * npair : 1024 * npair + 512].reshape(512, 2, 128)
            xtc[:, 2048 * npair :] = L.transpose(2, 1, 0).reshape(128, 1024)

        wh = np.zeros((128, WT_COLS), np.float16)
        wh[:, 0:64] = W16[0][e, 0:128, :]
        wh[:, 64:128] = W16[0][e, 128:256, :]
        for li in range(4):
            wc = 128 + 128 * li
            wh[0:64, wc : wc + 64] = W16[li + 1][e]
            wh[64:128, wc + 64 : wc + 128] = W16[li + 1][e]
        wh[0:64, W6C : W6C + A] = W16[5][e]
        wh[64:128, W6C + A : W6C + 2 * A] = W16[5][e]

        bh = np.zeros((128, 6), np.float32)
        for li in range(5):
            bh[0:64, li] = bsf[li][e]
            bh[64:128, li] = bsf[li][e]
        bh[0:A, 5] = bsf[5][e]
        bh[A : 2 * A, 5] = bsf[5][e]

        in_maps.append({"xt": xtc, "wt": wh, "bias": bh})

    meta = dict(
        B=B, nb=nb, npair=npair, nlone=nlone, C=C, counts=counts, csum=csum,
        order=order,
    )
    return in_maps, meta


def _finalize(results, meta):
    B, npair, nlone, C = (meta[k] for k in ("B", "npair", "nlone", "C"))
    counts, csum, order = meta["counts"], meta["csum"], meta["order"]
    y = np.empty((B, A), np.float32)
    for e in range(E):
        ytc = results[e]["yt"].astype(np.float32)  # [36, NU*512]; unit u at 512u
        rows = np.empty((C, A), np.float32)
        if npair:
            yp = ytc[:, 512 * nlone :].reshape(2, A, npair, 512)
            rows[: 1024 * npair] = (
                yp.transpose(2, 0, 3, 1).reshape(1024 * npair, A)
            )
        if nlone:
            rows[1024 * npair : 1024 * npair + 512] = ytc[0:A, 0:512].T
        y[order[csum[e] : csum[e + 1]]] = rows[: counts[e]]
    return y


def kernel(state, rm_state, W1, b1, W2, b2, W3, b3, W4, b4, W5, b5, W6, b6):
    global LAST_RESULTS
    from concourse.bass_utils import run_bass_kernel_spmd

    in_maps, meta = _prepare(
        state, rm_state, (W1, W2, W3, W4, W5, W6), (b1, b2, b3, b4, b5, b6)
    )
    nc = _get_program(meta["nb"])
    trace = bool(os.environ.get("KERNEL_TRACE"))
    res = run_bass_kernel_spmd(nc, in_maps, core_ids=list(range(NCORES)), trace=trace)
    LAST_RESULTS = res
    return _finalize(res.results, meta)
